# revision 14
# baseline (speedup 1.0000x reference)
"""Trainium2 Bass kernel for nn_DiscriminationLoss (segment_reduce).

Strategy (8 NeuronCores, pixel-sharded, one-hot + PE segment-sum):
  - Each core gets 1/8 of the 4M pixels: pred slice [8, 524288] f32 and
    labels slice [524288] i32, tiled [128 partitions x 4096 block-cols].
  - One-hot generation is the DVE bottleneck. tensor_tensor(is_equal)
    caps at the DVE 2x_1p perf mode (two-tensor ops can't use both read
    ports on cayman), so the one-hot is built with 32 per-k
    tensor_scalar(is_equal, imm=k) ops per chunk instead: single-source
    ops qualify for the 4x_2p mode (both read ports + both write ports,
    2 packed bf16 per port).  Measured on HW: FD=512 per-k op = ~194 ns
    -> 6.2 us per 512-block chunk vs 8.6 us for the equivalent TT.
  - One-hot layout is k-major: oh[p, j*CH + t] (j = label-1, t = block
    col in chunk).  Per-k writes are fully contiguous; the matmul reads
    a 2D moving AP [j:32 stride CH, b:8 stride 1] per tg group (256
    cols), accumulating psum[72, 256] over all 512 matmuls -- psum col
    = j*8+b, identical to the old (tg, j, b) layout, so the host-side
    extraction is unchanged.
  - pred is scaled by 2^14 and cast to fp16 on ScalarE (scale rides the
    activation's free affine); the ones column for counts is written by
    a second activation with scale=0, bias=1 from a memset tile.  Casts
    are issued at half-chunk granularity so the first matmuls are not
    gated on a full 2 MiB pred DMA + full-chunk cast.
  - PE warmup: ~30 dense matmuls on a memset tile (no DMA dependency)
    trip the HAM clock gate to 2.4 GHz before the real stream starts.
  - GpSimd only issues the labels' int32->bf16 cast-DMA (SWDGE); labels
    chunk 0 is issued first so the DVE stream starts ASAP.
  - Each core emits [128, 256] (PSUM readout + warmup dump row).  Host
    sums partials over cores and evaluates the tiny O(K^2) pairwise
    tail in f64.
"""

import sys
import functools

sys.path.insert(0, "/opt/trn_rl_repo")

import numpy as np

C = 8
K = 32
NCORES = 8
H = W = 2048
PTOT = H * W
PCORE = PTOT // NCORES  # 524288
SIGMA_DIS = 3.0
PRED_SCALE = float(2.0**14)

CH = 512   # max block-cols per chunk (tile allocation size)
CHUNKS = (256, 512, 512, 512, 512, 512, 512, 512, 256)  # sum = 4096
HC = 256   # block-cols per pred DMA / ACT cast half-chunk
QB = 8     # pixel-blocks batched per matmul (block-diagonal trick)
WARM_MMS = 32  # PE warmup matmuls (trip the HAM clock gate to 2.4 GHz)


def build_nc(pcore=PCORE, ch=CH, hc=HC, qb=QB, warm=WARM_MMS, chunks=CHUNKS):
    import concourse.bacc as bacc
    import concourse.tile as tile
    import concourse.mybir as mybir
    from contextlib import ExitStack

    ftot = pcore // 128
    assert pcore % 128 == 0
    if sum(chunks) != ftot:
        chunks = tuple([ch] * (ftot // ch))
    assert sum(chunks) == ftot and all(c % hc == 0 or c == hc for c in chunks)
    assert ch % hc == 0 and hc % qb == 0
    nchunk = len(chunks)
    f32 = mybir.dt.float32
    bf16 = mybir.dt.bfloat16
    fp16 = mybir.dt.float16
    i32 = mybir.dt.int32

    nch = C + 1
    ones_col = C
    tg_per_ch = ch // qb

    nc = bacc.Bacc(
        "TRN2", target_bir_lowering=False, debug=False, num_devices=NCORES
    )
    pred_ext = nc.dram_tensor("pred", [C, pcore], f32, kind="ExternalInput")
    lab_ext = nc.dram_tensor("labels", [pcore], i32, kind="ExternalInput")
    # rows 0..nch*qb-1: results; row 96: warmup dump (keeps warm MMs live)
    out_ext = nc.dram_tensor("out_s", [128, K * qb], f32, kind="ExternalOutput")

    with tile.TileContext(nc) as tc, ExitStack() as ctx:
        const_pool = ctx.enter_context(tc.tile_pool(name="const", bufs=1))
        slab32_pool = ctx.enter_context(tc.tile_pool(name="slab32", bufs=4))
        slabh_pool = ctx.enter_context(tc.tile_pool(name="slabh", bufs=4))
        labb_pool = ctx.enter_context(tc.tile_pool(name="labb", bufs=3))
        lab32_pool = ctx.enter_context(tc.tile_pool(name="lab32", bufs=3))
        oh_pool = ctx.enter_context(tc.tile_pool(name="oh", bufs=3))
        psum_pool = ctx.enter_context(tc.tile_pool(name="psum", bufs=1, space="PSUM"))
        out_pool = ctx.enter_context(tc.tile_pool(name="outp", bufs=1))

        # warm_t: warmup matmul operand + always-ready ACT input for the
        # ones column.  memset (no DMA) so the PE warmup starts with the
        # kernel body.
        warm_t = const_pool.tile([128, 256], bf16)
        nc.vector.memset(warm_t[:], 1.0)

        psum_full = psum_pool.tile([128, K * qb], f32)
        psum_t = psum_full[: nch * qb, :]

        # PE warmup: dense matmuls so the HAM clock gate opens before the
        # real stream (otherwise matmuls run at the cold 1.2 GHz rate).
        warm_ps = psum_pool.tile([128, 256], f32)
        if warm:
            for w in range(warm):
                nc.tensor.matmul(
                    warm_ps[:],
                    warm_t[:, :128],
                    warm_t[:, :256],
                    start=(w == 0),
                    stop=(w == warm - 1),
                )

        # Labels pixel mapping matches the pred half-chunk DMAs:
        # pixel(p, h, f) = base + 128*h*hc + p*hc + f, i.e. src AP
        # "(h p f) -> p h f".  All labels go over the sync HWDGE queue as
        # raw int32 (each chunk's labels DMA is issued just before that
        # chunk's pred DMAs, so chunk 0's labels win the DMA-engine
        # arbitration; SWDGE would add ~4 us of software-DGE latency) and
        # are cast to bf16 on the DVE right before the chunk's one-hot
        # ops (~0.6 us per chunk, hidden under the PE-bound steady state).
        nmm = ftot // qb  # total real matmuls
        mm = 0
        bases = [sum(chunks[:i]) for i in range(nchunk)]

        def lab_dma(ci):
            chc = chunks[ci]
            nh = max(1, chc // hc)
            base = bases[ci]
            lab32 = lab32_pool.tile([128, ch], i32, tag="lab32")
            nc.sync.dma_start(
                lab32[:, :chc].rearrange("p (h f) -> p h f", h=nh),
                lab_ext[128 * base : 128 * (base + chc)].rearrange(
                    "(h p f) -> p h f", h=nh, p=128
                ),
            )
            return lab32

        def lab_cast(ci, lab32):
            chc = chunks[ci]
            lbt = labb_pool.tile([128, ch], bf16, tag="labb")
            nc.vector.tensor_copy(lbt[:, :chc], lab32[:, :chc])
            return lbt

        lbts = {0: lab_cast(0, lab_dma(0))}
        for ci in range(nchunk):
            chc = chunks[ci]
            base = bases[ci]
            lbt = lbts.pop(ci)

            # Stationary tiles: tile-granular deps mean a matmul waits for
            # ALL writes to the tile it reads.  Chunk 0's stationary is
            # split per half-chunk so its first matmuls gate on half 0's
            # cast only; later chunks use one tile per chunk (their casts
            # finish well ahead) to halve the boundary-sem hiccups.
            split = ci == 0 and chc > hc
            nparts = (chc // hc) if split else 1
            pcols = chc // nparts
            slabhs = []
            for h in range(nparts):
                poff = 128 * (base + h * pcols)
                slab32 = slab32_pool.tile([128, C * ch], f32, tag="slab32")
                # pred DMA stays at half-chunk granularity (pixel mapping)
                for hh in range(max(1, pcols // hc)):
                    hcc = min(hc, pcols)
                    gpx = 128 * hcc
                    nc.sync.dma_start(
                        slab32[
                            :, C * hh * hcc : C * (hh + 1) * hcc
                        ].rearrange("p (c f) -> p c f", c=C),
                        pred_ext[
                            :, poff + 128 * hh * hcc : poff + 128 * hh * hcc + gpx
                        ].rearrange("c (p f) -> p c f", p=128),
                    )
                slabh = slabh_pool.tile([128, nch * ch], fp16, tag="slabh")
                slabh_r = slabh[:, : nch * pcols].rearrange(
                    "p (tg c b) -> p tg c b", c=nch, b=qb
                )  # [128, pcols//qb, nch, qb]
                for hh in range(max(1, pcols // hc)):
                    hcc = min(hc, pcols)
                    tg0 = hh * (hcc // qb)
                    tg1 = tg0 + hcc // qb
                    slab32_r = slab32[
                        :, C * hh * hcc : C * (hh + 1) * hcc
                    ].rearrange("p (c tg b) -> p tg c b", c=C, b=qb)
                    # scaled fp16 cast on ScalarE: out = Copy(in * 2^14)
                    nc.scalar.activation(
                        slabh_r[:, tg0:tg1, :C, :],
                        slab32_r,
                        mybir.ActivationFunctionType.Copy,
                        scale=PRED_SCALE,
                    )
                    # ones column via ACT: Copy(0*x + 1) = 1.0
                    nc.scalar.activation(
                        slabh_r[:, tg0:tg1, ones_col, :],
                        warm_t[:, :1].unsqueeze(2).broadcast_to(
                            [128, hcc // qb, qb]
                        ),
                        mybir.ActivationFunctionType.Copy,
                        bias=1.0,
                        scale=0.0,
                    )
                slabhs.append(slabh)

            # one-hot, k-major: oh[p, j*chc + t] = (labels[p, t] == j+1).
            # 32 single-source tensor_scalar ops -> DVE 4x_2p perf mode.
            oh = oh_pool.tile([128, K * ch], fp16, tag="oh")
            oh_r = oh[:, : K * chc].rearrange("p (j t) -> p j t", j=K)
            for j in range(K):
                if j == K // 2 and ci + 1 < nchunk:
                    # next chunk's labels DMA (sync queue, after this
                    # chunk's pred descs) + bf16 cast, placed mid-stream
                    # so no cast ever sits between a chunk's last one-hot
                    # op and its matmuls' semaphore gate.
                    lbts[ci + 1] = lab_cast(ci + 1, lab_dma(ci + 1))
                nc.vector.tensor_scalar(
                    oh_r[:, j, :],
                    lbt[:, :chc],
                    float(j + 1),
                    None,
                    mybir.AluOpType.is_equal,
                )

            oh2d = oh[:, : K * chc].rearrange(
                "p (j tg b) -> p j tg b", j=K, b=qb
            )
            tg_per_part = pcols // qb
            for tg in range(chc // qb):
                sh = slabhs[min(tg // tg_per_part, len(slabhs) - 1)]
                tgl = tg % tg_per_part
                nc.tensor.matmul(
                    psum_t[:],
                    sh[:, tgl * nch * qb : (tgl + 1) * nch * qb],
                    oh2d[:, :, tg, :],
                    start=(mm == 0),
                    stop=(mm == nmm - 1),
                )
                mm += 1

        outt = out_pool.tile([128, K * qb], f32)
        nc.vector.memset(outt[:], 0.0)
        nc.vector.tensor_copy(outt[: nch * qb, :], psum_t[:])
        if warm:
            nc.vector.tensor_copy(outt[96:97, :], warm_ps[96:97, : K * qb])
        nc.sync.dma_start(out_ext[:], outt[:])
    nc.compile()
    return nc


@functools.lru_cache(maxsize=1)
def _get_program():
    return build_nc()


def make_in_maps(pred_flat, labels_flat):
    in_maps = []
    for i in range(NCORES):
        sl = slice(i * PCORE, (i + 1) * PCORE)
        in_maps.append(
            {
                "pred": np.ascontiguousarray(pred_flat[:, sl]),
                "labels": np.ascontiguousarray(labels_flat[sl]),
            }
        )
    return in_maps


def finish_host(parts, num_kernel, qb=QB):
    """parts: per-core [128, K*qb] partials. Tiny O(K^2) tail in f64."""
    nch = C + 1
    total = np.sum([p.astype(np.float64) for p in parts], axis=0)
    r = total[: nch * qb, :].reshape(nch, qb, K, qb)
    total = r[:, np.arange(qb), :, np.arange(qb)].sum(axis=0)  # [nch, K]
    S = total[:C, :] / PRED_SCALE  # [8, 32]
    N = total[C, :]  # [32]
    A = N * np.sum(S * S, axis=0)  # [32]
    kk = int(num_kernel)
    A = A[:kk]
    pair = A[:, None] + A[None, :]
    Dm = np.maximum(SIGMA_DIS - np.sqrt(pair), 0.0)
    term = np.log(Dm * Dm + 1.0)
    L = float(np.sum(np.triu(term, k=1)))
    L *= (kk - 1) / kk
    return np.float32(L)


_last_results = None


def kernel(pred_similarities, regions_mask, kernel_labels, num_kernel, **kw):
    global _last_results
    from concourse.bass_utils import run_bass_kernel_spmd

    pred_flat = np.asarray(pred_similarities, dtype=np.float32).reshape(C, PTOT)
    labels_flat = np.asarray(kernel_labels, dtype=np.int32).reshape(PTOT)

    nc = _get_program()
    in_maps = make_in_maps(pred_flat, labels_flat)
    res = run_bass_kernel_spmd(nc, in_maps, list(range(NCORES)))
    _last_results = res
    parts = [res.results[i]["out_s"] for i in range(NCORES)]
    return finish_host(parts, num_kernel)
